# revision 4
# baseline (speedup 1.0000x reference)
"""COINBlock (retention/decay-masked attention) Trainium2 Bass kernel.

Sharding: 8 cores = (B=2) x (4-way sequence split of T=4096).
Each core computes its 1024-row chunk of decay-masked attention using a
chunked-recurrence decomposition:
  out[n] = Q[n] @ S_prefix * gamma^{n_loc}  (cross, via Gram-sandwich state)
         + sum_{m<=n, same chunk} gamma^{n-m} (Q K^T)[n,m] V[m]  (intra)
with S_prefix = W_K^T (Xp^T Gamma Xp) W_V computed locally (no collectives).
Output is produced transposed ([v, n]) so the reference's swapaxes+reshape
becomes a host-side concat+reshape.

SBUF is managed as one arena pool of 42 4KB-per-partition slots with manual
slot assignment per phase (Tile tags serialize reuse safely):
  A : G->0-7, xr stream->8-10, xs stream->11-13
  B1: wv->16-23, T2->24-31 (G read)
  B2: wk->8-15, S->32-39 (T2 read)
  C1: xt->0-7, V stage->40-41 (wv read; V bounced to DRAM)
  C2: Kst->16-23, drowinv->24 (wk read)
  C3: wq->8-15, drow->25, Qst->26-31,40,41 (xt read)
  D : V reload->8-15, A tiles->0-7 (S, Kst, Qst read)
"""
import sys
import numpy as np

sys.path.insert(0, "/opt/trn_rl_repo")

GAMMA = 0.99
B, T, I, C = 2, 4096, 1024, 1024
CHUNK = 1024          # rows of T per core
NCORES = 8
PREF = 3072           # max prefix rows any core needs for the Gram
NMC = PREF // 128     # 24 m-chunks
NB = I // 128         # 8 column blocks
H = 512               # matmul moving free dim (fp32 PSUM bank limit)

_cache = {}


def _build_nc():
    from contextlib import ExitStack
    from concourse import bacc, tile
    from concourse.bass import mybir

    f32 = mybir.dt.float32
    f32r = mybir.dt.float32r

    nc = bacc.Bacc("TRN2", target_bir_lowering=False, debug=False,
                   num_devices=NCORES)
    xp = nc.dram_tensor("xp", [PREF, I], f32, kind="ExternalInput").ap()
    xt = nc.dram_tensor("xt", [I, CHUNK], f32, kind="ExternalInput").ap()
    wq = nc.dram_tensor("wq", [I, C], f32, kind="ExternalInput").ap()
    wk = nc.dram_tensor("wk", [I, C], f32, kind="ExternalInput").ap()
    wv = nc.dram_tensor("wv", [I, I], f32, kind="ExternalInput").ap()
    dpref = nc.dram_tensor("dpref", [128, NMC], f32, kind="ExternalInput").ap()
    drow = nc.dram_tensor("drow", [128, CHUNK], f32, kind="ExternalInput").ap()
    drowinv = nc.dram_tensor("drowinv", [128, CHUNK], f32,
                             kind="ExternalInput").ap()
    dmask = nc.dram_tensor("dmask", [128, 4 * H], f32, kind="ExternalInput").ap()
    out = nc.dram_tensor("out", [I, CHUNK], f32, kind="ExternalOutput").ap()
    vscr = nc.dram_tensor("vscr", [CHUNK, I], f32).ap()  # V bounce buffer

    def blk(ap_, i):
        return ap_[:, i * 128:(i + 1) * 128]

    def half(ap_, h):
        return ap_[:, h * H:(h + 1) * H]

    with tile.TileContext(nc) as tc, ExitStack() as top:
        const = top.enter_context(tc.tile_pool(name="const", bufs=1))
        arena = top.enter_context(tc.tile_pool(name="arena", bufs=1))
        ost = top.enter_context(tc.tile_pool(name="ostage", bufs=3))

        def slot(i, shape, dtype):
            return arena.tile(shape, dtype, tag=f"s{i:02d}", name=f"s{i:02d}")

        t_dpref = const.tile([128, NMC], f32, tag="dpref", name="dpref")
        nc.sync.dma_start(t_dpref[:], dpref[:, :])
        t_dmask = const.tile([128, 4 * H], f32, tag="dmask", name="dmask")
        nc.sync.dma_start(t_dmask[:], dmask[:, :])

        # ---------------- Phase A: Gram G = (Gamma Xp)^T Xp ----------------
        g_tiles = [None] * NB
        with tc.tile_pool(name="gpsum", bufs=1, space="PSUM") as gps:
            for gpass in range(2):          # 4 a-blocks per pass (8 PSUM banks)
                pg = [gps.tile([128, I], f32, tag=f"pg{a}", name=f"pg{a}")
                      for a in range(4)]
                for mc in range(NMC):
                    xr = slot(8 + mc % 3, [128, I], f32r)
                    nc.sync.dma_start(
                        xr[:], xp[mc * 128:(mc + 1) * 128, :].bitcast(f32r))
                    xs = slot(11 + mc % 3, [128, I], f32r)
                    nc.vector.tensor_scalar_mul(
                        xs[:], xr[:], t_dpref[:, mc:mc + 1])
                    for a4 in range(4):
                        ablk = gpass * 4 + a4
                        for h in range(2):
                            nc.tensor.matmul(
                                half(pg[a4], h), blk(xs, ablk), half(xr, h),
                                start=(mc == 0), stop=(mc == NMC - 1))
                for a4 in range(4):
                    ablk = gpass * 4 + a4
                    gt = slot(ablk, [128, I], f32r)
                    nc.vector.tensor_copy(gt[:], pg[a4][:])
                    g_tiles[ablk] = gt

        # ---------------- Phase B1: T2 = G @ W_V (G symmetric) -------------
        wv_tiles = []
        for ib in range(NB):
            wt = slot(16 + ib, [128, I], f32r)
            nc.sync.dma_start(wt[:], wv[ib * 128:(ib + 1) * 128, :].bitcast(f32r))
            wv_tiles.append(wt)

        t2_tiles = []
        with tc.tile_pool(name="t2psum", bufs=2, space="PSUM") as tps:
            for ib in range(NB):
                pt = tps.tile([128, I], f32, tag="pt", name="pt")
                for bb in range(NB):
                    for h in range(2):
                        nc.tensor.matmul(
                            half(pt, h), blk(g_tiles[bb], ib),
                            half(wv_tiles[bb], h),
                            start=(bb == 0), stop=(bb == NB - 1))
                t2 = slot(24 + ib, [128, I], f32r)
                nc.vector.tensor_copy(t2[:], pt[:])
                t2_tiles.append(t2)

        # ---------------- Phase B2: S = W_K^T @ T2 -------------------------
        wk_tiles = []
        for ib in range(NB):
            wt = slot(8 + ib, [128, I], f32r)
            nc.sync.dma_start(wt[:], wk[ib * 128:(ib + 1) * 128, :].bitcast(f32r))
            wk_tiles.append(wt)

        s_tiles = []
        with tc.tile_pool(name="spsum", bufs=2, space="PSUM") as sps:
            for cb in range(NB):
                ps = sps.tile([128, I], f32, tag="ps", name="ps")
                for ib in range(NB):
                    for h in range(2):
                        nc.tensor.matmul(
                            half(ps, h), blk(wk_tiles[ib], cb),
                            half(t2_tiles[ib], h),
                            start=(ib == 0), stop=(ib == NB - 1))
                st = slot(32 + cb, [128, I], f32r)
                nc.vector.tensor_copy(st[:], ps[:])
                s_tiles.append(st)

        # ---------------- Phase C0: load Xc^T ------------------------------
        xt_tiles = []
        for ib in range(NB):
            xtt = slot(ib, [128, CHUNK], f32r)
            nc.sync.dma_start(xtt[:], xt[ib * 128:(ib + 1) * 128, :].bitcast(f32r))
            xt_tiles.append(xtt)

        # ---------------- Phase C1: V = Xc @ W_V -> DRAM bounce ------------
        with tc.tile_pool(name="vpsum", bufs=2, space="PSUM") as vps:
            for mb in range(NB):
                pv = vps.tile([128, I], f32, tag="pv", name="pv")
                for ib in range(NB):
                    for h in range(2):
                        nc.tensor.matmul(
                            half(pv, h), blk(xt_tiles[ib], mb),
                            half(wv_tiles[ib], h),
                            start=(ib == 0), stop=(ib == NB - 1))
                vt = slot(40 + mb % 2, [128, I], f32)
                nc.vector.tensor_copy(vt[:], pv[:])
                nc.sync.dma_start(vscr[mb * 128:(mb + 1) * 128, :], vt[:])

        # ---------------- Phase C2: Kst = (W_K^T Xc^T) * gamma^{-m} --------
        t_drowinv = slot(24, [128, CHUNK], f32)
        nc.sync.dma_start(t_drowinv[:], drowinv[:, :])
        kst_tiles = []
        with tc.tile_pool(name="kpsum", bufs=2, space="PSUM") as kps:
            for cb in range(NB):
                pk = kps.tile([128, CHUNK], f32, tag="pk", name="pk")
                for ib in range(NB):
                    for h in range(2):
                        nc.tensor.matmul(
                            half(pk, h), blk(wk_tiles[ib], cb),
                            half(xt_tiles[ib], h),
                            start=(ib == 0), stop=(ib == NB - 1))
                kt = slot(16 + cb, [128, CHUNK], f32r)
                nc.vector.tensor_mul(kt[:], pk[:], t_drowinv[:])
                kst_tiles.append(kt)

        # ---------------- Phase C3: Qst = (W_Q^T Xc^T) * gamma^{n} ---------
        t_drow = slot(25, [128, CHUNK], f32)
        nc.sync.dma_start(t_drow[:], drow[:, :])
        wq_tiles = []
        for ib in range(NB):
            wt = slot(8 + ib, [128, I], f32r)
            nc.sync.dma_start(wt[:], wq[ib * 128:(ib + 1) * 128, :].bitcast(f32r))
            wq_tiles.append(wt)
        QSLOTS = [26, 27, 28, 29, 30, 31, 40, 41]
        qst_tiles = []
        with tc.tile_pool(name="qpsum", bufs=2, space="PSUM") as qps:
            for cb in range(NB):
                pq = qps.tile([128, CHUNK], f32, tag="pq", name="pq")
                for ib in range(NB):
                    for h in range(2):
                        nc.tensor.matmul(
                            half(pq, h), blk(wq_tiles[ib], cb),
                            half(xt_tiles[ib], h),
                            start=(ib == 0), stop=(ib == NB - 1))
                qt = slot(QSLOTS[cb], [128, CHUNK], f32r)
                nc.vector.tensor_mul(qt[:], pq[:], t_drow[:])
                qst_tiles.append(qt)

        # ---------------- Phase D: reload V, attention ---------------------
        v_tiles = []
        for mb in range(NB):
            vt = slot(8 + mb, [128, I], f32r)
            nc.sync.dma_start(vt[:], vscr[mb * 128:(mb + 1) * 128, :].bitcast(f32r))
            v_tiles.append(vt)

        with tc.tile_pool(name="apsum", bufs=3, space="PSUM") as aps, \
             tc.tile_pool(name="opsum", bufs=3, space="PSUM") as ops:
            for nh in range(2):
                n_mblks = 4 if nh == 0 else NB
                a_tiles = []
                for mb in range(n_mblks):
                    pa = aps.tile([128, H], f32, tag="pa", name="pa")
                    for cb in range(NB):
                        nc.tensor.matmul(
                            pa[:], blk(kst_tiles[cb], mb),
                            half(qst_tiles[cb], nh),
                            start=(cb == 0), stop=(cb == NB - 1))
                    at = slot(mb, [128, H], f32r)
                    nc.vector.tensor_copy(at[:], pa[:])
                    # causal fixup: zero/mask where global m > n
                    d0 = mb * 128 - nh * H
                    if 0 <= d0 < H:
                        case = d0 // 128
                        nc.vector.tensor_mul(
                            at[:], at[:], t_dmask[:, case * H:(case + 1) * H])
                    a_tiles.append(at)
                for vb in range(NB):
                    po = ops.tile([128, H], f32, tag="po", name="po")
                    for cb in range(NB):
                        nc.tensor.matmul(
                            po[:], blk(s_tiles[cb], vb), half(qst_tiles[cb], nh),
                            start=(cb == 0), stop=False)
                    for k, mb in enumerate(range(n_mblks)):
                        nc.tensor.matmul(
                            po[:], blk(v_tiles[mb], vb), a_tiles[mb][:],
                            start=False, stop=(k == n_mblks - 1))
                    ob = ost.tile([128, H], f32, tag="ob", name="ob")
                    nc.vector.tensor_copy(ob[:], po[:])
                    nc.sync.dma_start(
                        out[vb * 128:(vb + 1) * 128, nh * H:(nh + 1) * H], ob[:])

    nc.compile()
    return nc


def _host_inputs(X, W_Q, W_K, W_V):
    n_loc = np.arange(CHUNK, dtype=np.float64)
    drow_v = (GAMMA ** n_loc).astype(np.float32)
    drowinv_v = (GAMMA ** (-n_loc)).astype(np.float32)
    drow_t = np.ascontiguousarray(np.broadcast_to(drow_v[None, :], (128, CHUNK)))
    drowinv_t = np.ascontiguousarray(
        np.broadcast_to(drowinv_v[None, :], (128, CHUNK)))
    # 4 causal mask variants: mask[p, f] = 1 if f >= p + case*128
    dmask_t = np.concatenate(
        [(np.arange(H)[None, :] >= (np.arange(128)[:, None] + case * 128))
         .astype(np.float32) for case in range(4)], axis=1)
    dmask_t = np.ascontiguousarray(dmask_t)
    wq = np.ascontiguousarray(W_Q, dtype=np.float32)
    wk = np.ascontiguousarray(W_K, dtype=np.float32)
    wv = np.ascontiguousarray(W_V, dtype=np.float32)
    in_maps = []
    for core in range(NCORES):
        b, j = divmod(core, 4)
        r0 = j * CHUNK
        m = np.arange(PREF, dtype=np.float64)
        dp = np.where(m < r0, GAMMA ** (r0 - m), 0.0).astype(np.float32)
        dp_t = np.ascontiguousarray(dp.reshape(NMC, 128).T)
        in_maps.append({
            "xp": np.ascontiguousarray(X[b, :PREF], dtype=np.float32),
            "xt": np.ascontiguousarray(X[b, r0:r0 + CHUNK].T, dtype=np.float32),
            "wq": wq, "wk": wk, "wv": wv,
            "dpref": dp_t, "drow": drow_t, "drowinv": drowinv_t,
            "dmask": dmask_t,
        })
    return in_maps


def run_on_device(X, W_Q, W_K, W_V, trace=False, trace_cores=None):
    from concourse import bass_utils
    if "nc" not in _cache:
        _cache["nc"] = _build_nc()
    nc = _cache["nc"]
    in_maps = _host_inputs(X, W_Q, W_K, W_V)
    res = bass_utils.run_bass_kernel_spmd(
        nc, in_maps, core_ids=list(range(NCORES)), trace=trace,
        trace_cores=trace_cores)
    outT = np.empty((B, I, T), dtype=np.float32)
    for core in range(NCORES):
        b, j = divmod(core, 4)
        outT[b][:, j * CHUNK:(j + 1) * CHUNK] = res.results[core]["out"]
    out = outT.reshape(B, T, C)
    return out, res


def kernel(X, att_mask, S_n, W_Q, W_K, W_V):
    X = np.asarray(X, dtype=np.float32)
    out, _ = run_on_device(X, np.asarray(W_Q), np.asarray(W_K), np.asarray(W_V))
    return out, np.asarray(S_n)


# revision 5
# speedup vs baseline: 155.4096x; 155.4096x over previous
"""COINBlock (retention/decay-masked attention) Trainium2 Bass kernel.

Sharding: 8 cores = (B=2) x (4-way sequence split of T=4096).
Each core computes its 1024-row chunk of decay-masked attention using a
chunked-recurrence decomposition:
  out[n] = Q[n] @ S_prefix * gamma^{n_loc}  (cross, via Gram-sandwich state)
         + sum_{m<=n, same chunk} gamma^{n-m} (Q K^T)[n,m] V[m]  (intra)
with S_prefix = W_K^T (Xp^T Gamma Xp) W_V computed locally (no collectives).
Output is produced transposed ([v, n]) so the reference's swapaxes+reshape
becomes a host-side concat+reshape.

SBUF is managed as one arena pool of 42 4KB-per-partition slots with manual
slot assignment per phase (Tile tags serialize reuse safely):
  A : G->0-7, xr stream->8-10, xs stream->11-13
  B1: wv->16-23, T2->24-31 (G read)
  B2: wk->8-15, S->32-39 (T2 read)
  C1: xt->0-7, V stage->40-41 (wv read; V bounced to DRAM)
  C2: Kst->16-23, drowinv->24 (wk read)
  C3: wq->8-15, drow->25, Qst->26-31,40,41 (xt read)
  D : V reload->8-15, A tiles->0-7 (S, Kst, Qst read)
"""
import sys
import numpy as np

sys.path.insert(0, "/opt/trn_rl_repo")

GAMMA = 0.99
B, T, I, C = 2, 4096, 1024, 1024
CHUNK = 1024          # rows of T per core
NCORES = 8
PREF = 3072           # max prefix rows any core needs for the Gram
NMC = PREF // 128     # 24 m-chunks
NB = I // 128         # 8 column blocks
H = 512               # matmul moving free dim (fp32 PSUM bank limit)

_cache = {}


def _build_nc(reps=1):
    from contextlib import ExitStack
    from concourse import bacc, tile
    from concourse.bass import mybir

    f32 = mybir.dt.float32
    f32r = mybir.dt.float32r

    nc = bacc.Bacc("TRN2", target_bir_lowering=False, debug=False,
                   num_devices=NCORES)
    xp = nc.dram_tensor("xp", [PREF, I], f32, kind="ExternalInput").ap()
    xt = nc.dram_tensor("xt", [I, CHUNK], f32, kind="ExternalInput").ap()
    wq = nc.dram_tensor("wq", [I, C], f32, kind="ExternalInput").ap()
    wk = nc.dram_tensor("wk", [I, C], f32, kind="ExternalInput").ap()
    wv = nc.dram_tensor("wv", [I, I], f32, kind="ExternalInput").ap()
    dpref = nc.dram_tensor("dpref", [128, NMC], f32, kind="ExternalInput").ap()
    drow = nc.dram_tensor("drow", [128, CHUNK], f32, kind="ExternalInput").ap()
    drowinv = nc.dram_tensor("drowinv", [128, CHUNK], f32,
                             kind="ExternalInput").ap()
    dmask = nc.dram_tensor("dmask", [128, 4 * H], f32, kind="ExternalInput").ap()
    out = nc.dram_tensor("out", [I, CHUNK], f32, kind="ExternalOutput").ap()
    vscr = nc.dram_tensor("vscr", [CHUNK, I], f32).ap()  # V bounce buffer

    def blk(ap_, i):
        return ap_[:, i * 128:(i + 1) * 128]

    def half(ap_, h):
        return ap_[:, h * H:(h + 1) * H]

    with tile.TileContext(nc) as tc, ExitStack() as top:
        const = top.enter_context(tc.tile_pool(name="const", bufs=1))
        arena = top.enter_context(tc.tile_pool(name="arena", bufs=1))
        ost = top.enter_context(tc.tile_pool(name="ostage", bufs=3))

        def slot(i, shape, dtype):
            return arena.tile(shape, dtype, tag=f"s{i:02d}", name=f"s{i:02d}")

        t_dpref = const.tile([128, NMC], f32, tag="dpref", name="dpref")
        nc.sync.dma_start(t_dpref[:], dpref[:, :])
        t_dmask = const.tile([128, 4 * H], f32, tag="dmask", name="dmask")
        nc.sync.dma_start(t_dmask[:], dmask[:, :])

        for rep in range(reps):
            _emit_body(nc, tc, rep, slot, ost, t_dpref, t_dmask,
                       xp, xt, wq, wk, wv, drow, drowinv, out, vscr,
                       blk, half, f32, f32r)

    nc.compile()
    return nc


def _emit_body(nc, tc, rep, slot, ost, t_dpref, t_dmask,
               xp, xt, wq, wk, wv, drow, drowinv, out, vscr,
               blk, half, f32, f32r):
    if True:
        # ---------------- Phase A: Gram G = (Gamma Xp)^T Xp ----------------
        g_tiles = [None] * NB
        with tc.tile_pool(name=f"gpsum{rep}", bufs=1, space="PSUM") as gps:
            for gpass in range(2):          # 4 a-blocks per pass (8 PSUM banks)
                pg = [gps.tile([128, I], f32, tag=f"pg{a}", name=f"pg{a}")
                      for a in range(4)]
                for mc in range(NMC):
                    xr = slot(8 + mc % 3, [128, I], f32r)
                    nc.sync.dma_start(
                        xr[:], xp[mc * 128:(mc + 1) * 128, :].bitcast(f32r))
                    xs = slot(11 + mc % 3, [128, I], f32r)
                    nc.vector.tensor_scalar_mul(
                        xs[:], xr[:], t_dpref[:, mc:mc + 1])
                    for a4 in range(4):
                        ablk = gpass * 4 + a4
                        for h in range(2):
                            nc.tensor.matmul(
                                half(pg[a4], h), blk(xs, ablk), half(xr, h),
                                start=(mc == 0), stop=(mc == NMC - 1))
                for a4 in range(4):
                    ablk = gpass * 4 + a4
                    gt = slot(ablk, [128, I], f32r)
                    nc.vector.tensor_copy(gt[:], pg[a4][:])
                    g_tiles[ablk] = gt

        # ---------------- Phase B1: T2 = G @ W_V (G symmetric) -------------
        wv_tiles = []
        for ib in range(NB):
            wt = slot(16 + ib, [128, I], f32r)
            nc.sync.dma_start(wt[:], wv[ib * 128:(ib + 1) * 128, :].bitcast(f32r))
            wv_tiles.append(wt)

        t2_tiles = []
        with tc.tile_pool(name=f"t2psum{rep}", bufs=2, space="PSUM") as tps:
            for ib in range(NB):
                pt = tps.tile([128, I], f32, tag="pt", name="pt")
                for bb in range(NB):
                    for h in range(2):
                        nc.tensor.matmul(
                            half(pt, h), blk(g_tiles[bb], ib),
                            half(wv_tiles[bb], h),
                            start=(bb == 0), stop=(bb == NB - 1))
                t2 = slot(24 + ib, [128, I], f32r)
                nc.vector.tensor_copy(t2[:], pt[:])
                t2_tiles.append(t2)

        # ---------------- Phase B2: S = W_K^T @ T2 -------------------------
        wk_tiles = []
        for ib in range(NB):
            wt = slot(8 + ib, [128, I], f32r)
            nc.sync.dma_start(wt[:], wk[ib * 128:(ib + 1) * 128, :].bitcast(f32r))
            wk_tiles.append(wt)

        s_tiles = []
        with tc.tile_pool(name=f"spsum{rep}", bufs=2, space="PSUM") as sps:
            for cb in range(NB):
                ps = sps.tile([128, I], f32, tag="ps", name="ps")
                for ib in range(NB):
                    for h in range(2):
                        nc.tensor.matmul(
                            half(ps, h), blk(wk_tiles[ib], cb),
                            half(t2_tiles[ib], h),
                            start=(ib == 0), stop=(ib == NB - 1))
                st = slot(32 + cb, [128, I], f32r)
                nc.vector.tensor_copy(st[:], ps[:])
                s_tiles.append(st)

        # ---------------- Phase C0: load Xc^T ------------------------------
        xt_tiles = []
        for ib in range(NB):
            xtt = slot(ib, [128, CHUNK], f32r)
            nc.sync.dma_start(xtt[:], xt[ib * 128:(ib + 1) * 128, :].bitcast(f32r))
            xt_tiles.append(xtt)

        # ---------------- Phase C1: V = Xc @ W_V -> DRAM bounce ------------
        with tc.tile_pool(name=f"vpsum{rep}", bufs=2, space="PSUM") as vps:
            for mb in range(NB):
                pv = vps.tile([128, I], f32, tag="pv", name="pv")
                for ib in range(NB):
                    for h in range(2):
                        nc.tensor.matmul(
                            half(pv, h), blk(xt_tiles[ib], mb),
                            half(wv_tiles[ib], h),
                            start=(ib == 0), stop=(ib == NB - 1))
                vt = slot(40 + mb % 2, [128, I], f32)
                nc.vector.tensor_copy(vt[:], pv[:])
                nc.sync.dma_start(vscr[mb * 128:(mb + 1) * 128, :], vt[:])

        # ---------------- Phase C2: Kst = (W_K^T Xc^T) * gamma^{-m} --------
        t_drowinv = slot(24, [128, CHUNK], f32)
        nc.sync.dma_start(t_drowinv[:], drowinv[:, :])
        kst_tiles = []
        with tc.tile_pool(name=f"kpsum{rep}", bufs=2, space="PSUM") as kps:
            for cb in range(NB):
                pk = kps.tile([128, CHUNK], f32, tag="pk", name="pk")
                for ib in range(NB):
                    for h in range(2):
                        nc.tensor.matmul(
                            half(pk, h), blk(wk_tiles[ib], cb),
                            half(xt_tiles[ib], h),
                            start=(ib == 0), stop=(ib == NB - 1))
                kt = slot(16 + cb, [128, CHUNK], f32r)
                nc.vector.tensor_mul(kt[:], pk[:], t_drowinv[:])
                kst_tiles.append(kt)

        # ---------------- Phase C3: Qst = (W_Q^T Xc^T) * gamma^{n} ---------
        t_drow = slot(25, [128, CHUNK], f32)
        nc.sync.dma_start(t_drow[:], drow[:, :])
        wq_tiles = []
        for ib in range(NB):
            wt = slot(8 + ib, [128, I], f32r)
            nc.sync.dma_start(wt[:], wq[ib * 128:(ib + 1) * 128, :].bitcast(f32r))
            wq_tiles.append(wt)
        QSLOTS = [26, 27, 28, 29, 30, 31, 40, 41]
        qst_tiles = []
        with tc.tile_pool(name=f"qpsum{rep}", bufs=2, space="PSUM") as qps:
            for cb in range(NB):
                pq = qps.tile([128, CHUNK], f32, tag="pq", name="pq")
                for ib in range(NB):
                    for h in range(2):
                        nc.tensor.matmul(
                            half(pq, h), blk(wq_tiles[ib], cb),
                            half(xt_tiles[ib], h),
                            start=(ib == 0), stop=(ib == NB - 1))
                qt = slot(QSLOTS[cb], [128, CHUNK], f32r)
                nc.vector.tensor_mul(qt[:], pq[:], t_drow[:])
                qst_tiles.append(qt)

        # ---------------- Phase D: reload V, attention ---------------------
        v_tiles = []
        for mb in range(NB):
            vt = slot(8 + mb, [128, I], f32r)
            nc.sync.dma_start(vt[:], vscr[mb * 128:(mb + 1) * 128, :].bitcast(f32r))
            v_tiles.append(vt)

        with tc.tile_pool(name=f"apsum{rep}", bufs=3, space="PSUM") as aps, \
             tc.tile_pool(name=f"opsum{rep}", bufs=3, space="PSUM") as ops:
            for nh in range(2):
                n_mblks = 4 if nh == 0 else NB
                a_tiles = []
                for mb in range(n_mblks):
                    pa = aps.tile([128, H], f32, tag="pa", name="pa")
                    for cb in range(NB):
                        nc.tensor.matmul(
                            pa[:], blk(kst_tiles[cb], mb),
                            half(qst_tiles[cb], nh),
                            start=(cb == 0), stop=(cb == NB - 1))
                    at = slot(mb, [128, H], f32r)
                    nc.vector.tensor_copy(at[:], pa[:])
                    # causal fixup: zero/mask where global m > n
                    d0 = mb * 128 - nh * H
                    if 0 <= d0 < H:
                        case = d0 // 128
                        nc.vector.tensor_mul(
                            at[:], at[:], t_dmask[:, case * H:(case + 1) * H])
                    a_tiles.append(at)
                for vb in range(NB):
                    po = ops.tile([128, H], f32, tag="po", name="po")
                    for cb in range(NB):
                        nc.tensor.matmul(
                            po[:], blk(s_tiles[cb], vb), half(qst_tiles[cb], nh),
                            start=(cb == 0), stop=False)
                    for k, mb in enumerate(range(n_mblks)):
                        nc.tensor.matmul(
                            po[:], blk(v_tiles[mb], vb), a_tiles[mb][:],
                            start=False, stop=(k == n_mblks - 1))
                    ob = ost.tile([128, H], f32, tag="ob", name="ob")
                    nc.vector.tensor_copy(ob[:], po[:])
                    nc.sync.dma_start(
                        out[vb * 128:(vb + 1) * 128, nh * H:(nh + 1) * H], ob[:])


def _host_inputs(X, W_Q, W_K, W_V):
    n_loc = np.arange(CHUNK, dtype=np.float64)
    drow_v = (GAMMA ** n_loc).astype(np.float32)
    drowinv_v = (GAMMA ** (-n_loc)).astype(np.float32)
    drow_t = np.ascontiguousarray(np.broadcast_to(drow_v[None, :], (128, CHUNK)))
    drowinv_t = np.ascontiguousarray(
        np.broadcast_to(drowinv_v[None, :], (128, CHUNK)))
    # 4 causal mask variants: mask[p, f] = 1 if f >= p + case*128
    dmask_t = np.concatenate(
        [(np.arange(H)[None, :] >= (np.arange(128)[:, None] + case * 128))
         .astype(np.float32) for case in range(4)], axis=1)
    dmask_t = np.ascontiguousarray(dmask_t)
    wq = np.ascontiguousarray(W_Q, dtype=np.float32)
    wk = np.ascontiguousarray(W_K, dtype=np.float32)
    wv = np.ascontiguousarray(W_V, dtype=np.float32)
    in_maps = []
    for core in range(NCORES):
        b, j = divmod(core, 4)
        r0 = j * CHUNK
        m = np.arange(PREF, dtype=np.float64)
        dp = np.where(m < r0, GAMMA ** (r0 - m), 0.0).astype(np.float32)
        dp_t = np.ascontiguousarray(dp.reshape(NMC, 128).T)
        in_maps.append({
            "xp": np.ascontiguousarray(X[b, :PREF], dtype=np.float32),
            "xt": np.ascontiguousarray(X[b, r0:r0 + CHUNK].T, dtype=np.float32),
            "wq": wq, "wk": wk, "wv": wv,
            "dpref": dp_t, "drow": drow_t, "drowinv": drowinv_t,
            "dmask": dmask_t,
        })
    return in_maps


def run_on_device(X, W_Q, W_K, W_V, trace=False, trace_cores=None):
    from concourse import bass_utils
    if "nc" not in _cache:
        _cache["nc"] = _build_nc()
    nc = _cache["nc"]
    in_maps = _host_inputs(X, W_Q, W_K, W_V)
    res = bass_utils.run_bass_kernel_spmd(
        nc, in_maps, core_ids=list(range(NCORES)), trace=trace,
        trace_cores=trace_cores)
    outT = np.empty((B, I, T), dtype=np.float32)
    for core in range(NCORES):
        b, j = divmod(core, 4)
        outT[b][:, j * CHUNK:(j + 1) * CHUNK] = res.results[core]["out"]
    out = outT.reshape(B, T, C)
    return out, res


def kernel(X, att_mask, S_n, W_Q, W_K, W_V):
    X = np.asarray(X, dtype=np.float32)
    out, _ = run_on_device(X, np.asarray(W_Q), np.asarray(W_K), np.asarray(W_V))
    return out, np.asarray(S_n)
